# revision 58
# baseline (speedup 1.0000x reference)
"""Multi-head attention (dense_transformer) Trainium2 Bass kernel.

Problem: x[8, 512, 32, 32]; per-batch 1x1-conv QKV projections, 8-head
attention over N=H*W=1024 positions (head_dim 64), output projection,
residual. Sharding: data-parallel over batch B=8 across the 8 cores —
one batch element per core, no collectives.

Per-core dataflow (all matmul inputs bf16, accumulation fp32):
  - Host pre-transposes weights to [c, o] layout and pre-casts to bf16.
  - Q, K in [c, n] layout: Q[ot] = WqT[ct].T @ x16[ct] (+bq).
  - V kept transposed: VT[jt][n, o] = x16[:, jt].T @ WvT (+bv), stored
    per-head with a ones column appended: [128, 8 heads, 65].
  - S^T[j, i] = K_h.T Q_h per head: j on partitions -> AV matmul needs
    no transposes anywhere. exp via ScalarE with the 1/sqrt(64) scale
    folded in; softmax denominator comes from the VT ones column during
    the AV matmul (PSUM row 64); normalization = reciprocal + DRAM-
    bounce partition broadcast + VectorE multiply.
  - out = WoT.T @ O + (x32 + bo prefolded), DMA'd out in fp32.

PSUM (8 banks) is phase-scoped: projections use a 4-buf half-bank pool
that closes before the AV-accumulator pool opens in the same banks.
"""

import sys

if "/opt/trn_rl_repo" not in sys.path:
    sys.path.insert(0, "/opt/trn_rl_repo")

import numpy as np
import ml_dtypes

import concourse.bass as bass
import concourse.mybir as mybir
from concourse.tile import TileContext

DIM = 512
NH = 8
HD = 64
N = 1024
P = 128
CT = DIM // P  # 4 c-tiles of 128 channels
JT = N // P    # 8 j-tiles of 128 positions
F32 = mybir.dt.float32
BF16 = mybir.dt.bfloat16
AOP = mybir.AluOpType
EXP = mybir.ActivationFunctionType.Exp


class FixedTileContext(TileContext):
    """Works around a walrus/bass snapshot mismatch: this walrus build
    accepts only one sync-wait command per instruction, but Tile's wait
    assigner happily attaches several. After scheduling, excess waits on
    any instruction are peeled off onto same-engine NOPs inserted right
    before it (same blocking semantics: the engine executes in order)."""

    MAX_WAITS = 1
    MAX_WAITS_DATA = 1
    _wsplit_ctr = 0

    def _split_sync_waits(self):
        seq_only = mybir.SEQUENCER_ONLY_OPCODES
        for fn in self.nc.m.functions:
            for blk in fn.blocks:
                insts = list(blk.instructions)
                out = []
                for inst in insts:
                    si = inst.sync_info
                    limit = (
                        self.MAX_WAITS
                        if inst.opcode in seq_only
                        else self.MAX_WAITS_DATA
                    )
                    if si is not None and len(si.on_wait) > limit:
                        waits = list(si.on_wait)
                        movers = waits[:-limit]
                        keep = waits[-limit:]
                        del si.on_wait[:]
                        for w in keep:
                            si.on_wait.append(w)
                        for w in movers:
                            FixedTileContext._wsplit_ctr += 1
                            nop = mybir.InstNoOp(
                                name=f"wsplit-{FixedTileContext._wsplit_ctr}",
                                ins=[],
                                outs=[],
                            )
                            nop.engine = inst.engine
                            nop.sync_info = mybir.SyncInfo(on_wait=[w], on_update=[])
                            out.append(nop)
                    out.append(inst)
                if len(out) != len(insts):
                    del blk.instructions[:]
                    for i in out:
                        blk.add_instruction(i)

    split_on_exit = True

    def __exit__(self, *exc):
        ret = super().__exit__(*exc)
        if exc[0] is None and self.split_on_exit:
            self._split_sync_waits()
        return ret


def build_nc(split_waits=True):
    nc = bass.Bass()

    x32d = nc.dram_tensor("x32", [DIM, N], F32, kind="ExternalInput")
    x16d = nc.dram_tensor("x16", [DIM, N], BF16, kind="ExternalInput")
    wqd = nc.dram_tensor("wqt", [DIM, DIM], BF16, kind="ExternalInput")
    wkd = nc.dram_tensor("wkt", [DIM, DIM], BF16, kind="ExternalInput")
    wvd = nc.dram_tensor("wvt", [DIM, DIM], BF16, kind="ExternalInput")
    wod = nc.dram_tensor("wot", [DIM, DIM], BF16, kind="ExternalInput")
    bqd = nc.dram_tensor("bq", [DIM], F32, kind="ExternalInput")
    bkd = nc.dram_tensor("bk", [DIM], F32, kind="ExternalInput")
    bvd = nc.dram_tensor("bv", [DIM], F32, kind="ExternalInput")
    bod = nc.dram_tensor("bo", [DIM], F32, kind="ExternalInput")
    outd = nc.dram_tensor("out", [DIM, N], F32, kind="ExternalOutput")

    FixedTileContext.split_on_exit = split_waits
    with FixedTileContext(nc) as tc:
        with (
            tc.tile_pool(name="persist", bufs=1) as persist,
            tc.tile_pool(name="ppool", bufs=32) as ppool,
            tc.tile_pool(name="small", bufs=4) as small,
            tc.tile_pool(name="otile", bufs=4) as otile,
            tc.tile_pool(name="dram", bufs=1, space="DRAM") as dram,
            tc.tile_pool(name="psS", bufs=2, space="PSUM") as psS_pool,
        ):
            # weights/biases ride ScalarE's DMA queues (ScalarE is idle
            # until the first exp) so they don't serialize behind the x
            # loads on SP's queues
            def load_w(wd, name):
                wr = wd.rearrange("(t p) o -> t p o", p=P)
                ws = []
                for t in range(CT):
                    wt = persist.tile(
                        [P, DIM], BF16, tag=f"{name}_{t}", name=f"{name}_{t}"
                    )
                    nc.scalar.dma_start(out=wt, in_=wr[t])
                    ws.append(wt)
                return ws

            def load_b(bd, name):
                bt = persist.tile([P, CT], F32, tag=name, name=name)
                nc.scalar.dma_start(out=bt, in_=bd.rearrange("(t p) -> p t", p=P))
                return bt

            # S^T + exp for one head pair. Emission alternates PE row
            # groups 0-63 / 64-127 between consecutive matmuls so the
            # hardware overlaps them (per-subarray concurrency) even
            # though K=64 only half-fills the array.
            def s_phase(pair):
                P16 = {}
                for jt in range(JT):
                    tiles = {}

                    def smm(h2, ih):
                        base = 64 * h2
                        nc.tensor.matmul(
                            tiles[h2][:, ih * 512 : (ih + 1) * 512],
                            lhsT=K[pair][base : base + 64, jt * P : (jt + 1) * P],
                            rhs=Q[pair][base : base + 64, ih * 512 : (ih + 1) * 512],
                            start=True,
                            stop=True,
                        )

                    tiles[0] = psS_pool.tile([P, N], F32, tag="psS", name="psS")
                    smm(0, 0)
                    tiles[1] = psS_pool.tile([P, N], F32, tag="psS", name="psS")
                    smm(1, 0)
                    smm(0, 1)
                    smm(1, 1)
                    for h2 in range(2):
                        pt = ppool.tile([P, N], BF16, tag="p16", name="p16")
                        nc.scalar.activation(pt, tiles[h2], EXP, scale=0.125)
                        P16[(jt, h2)] = pt
                return P16

            def p16_slice(P16, jt, h2, ih):
                return P16[(jt, h2)][:, ih * 512 : (ih + 1) * 512]

            # AV matmul + softmax normalization for one head pair. The raw
            # head output is copied out of PSUM right away (frees the psO
            # slot for the next head's AV); the DRAM-bounce broadcast and
            # the normalize multiply then run off the critical PSUM path.
            def av_phase(pair, P16, psO_pool, O16, rdram):
                last_pair = pair == NH // 2 - 1
                h2_order = (1, 0) if last_pair else (0, 1)
                for h2 in h2_order:
                    h = 2 * pair + h2
                    rec = small.tile([HD + 1, N], F32, tag="rec", name="rec")
                    oraw = small.tile([HD, N], F32, tag="oraw", name="oraw")
                    rb = small.tile([HD, N], F32, tag="rb", name="rb")
                    for ih in range(2):
                        sl = slice(ih * 512, (ih + 1) * 512)
                        po = psO_pool.tile([HD + 1, 512], F32, tag="psO", name="po")
                        for jt in range(JT):
                            nc.tensor.matmul(
                                po,
                                lhsT=VT[jt][:, h, :],
                                rhs=p16_slice(P16, jt, h2, ih),
                                start=(jt == 0),
                                stop=(jt == JT - 1),
                            )
                        # softmax denominator sits in row HD of po
                        nc.vector.reciprocal(rec[HD : HD + 1, sl], po[HD : HD + 1, :])
                        # copy the raw head output out of PSUM immediately
                        # (frees the psO slot); on the last pair ScalarE is
                        # done with exps, so use it and keep DVE off the
                        # critical chain
                        if last_pair:
                            nc.scalar.copy(oraw[:, sl], po[0:HD, :])
                        else:
                            nc.vector.tensor_copy(oraw[:, sl], po[0:HD, :])
                        # per-half DRAM bounce broadcasts 1/colsum across
                        # partitions (SBUF APs reject 0 partition stride)
                        dmae = nc.scalar if last_pair else nc.sync
                        dmae.dma_start(
                            out=rdram[h : h + 1, sl], in_=rec[HD : HD + 1, sl]
                        )
                        rsrc = rdram[h : h + 1, sl]
                        nc.sync.dma_start(
                            out=rb[:, sl],
                            in_=bass.AP(
                                tensor=rsrc.tensor,
                                offset=rsrc.offset,
                                ap=[[0, HD]] + list(rsrc.ap[1:]),
                            ),
                        )
                    osc = None
                    if h2 != 0:
                        osc = small.tile([HD, N], BF16, tag="osc", name="osc")
                    for ih in range(2):
                        sl = slice(ih * 512, (ih + 1) * 512)
                        if h2 == 0:
                            nc.vector.tensor_tensor(
                                O16[pair][0:HD, sl], oraw[:, sl], rb[:, sl], AOP.mult
                            )
                        else:
                            nc.vector.tensor_tensor(
                                osc[:, sl], oraw[:, sl], rb[:, sl], AOP.mult
                            )
                            (nc.scalar if last_pair else nc.sync).dma_start(
                                out=O16[pair][HD:P, sl], in_=osc[:, sl]
                            )

            with tc.tile_pool(name="pp", bufs=4, space="PSUM") as pp:
                # ---------- input loads ----------
                x16r = x16d.rearrange("(t p) n -> t p n", p=P)
                xs16 = []
                for t in range(CT):
                    xt = persist.tile([P, N], BF16, tag=f"x16_{t}", name=f"x16_{t}")
                    nc.sync.dma_start(out=xt, in_=x16r[t])
                    xs16.append(xt)

                wqs = load_w(wqd, "wq")
                wks = load_w(wkd, "wk")
                bq_sb = load_b(bqd, "bq")
                bk_sb = load_b(bkd, "bk")

                # trigger the ~2.7us exp table load on ScalarE right after
                # its weight-DMA issues, so the first real exp doesn't pay it
                warm = small.tile([1, 8], F32, tag="warm", name="warm")
                nc.vector.memset(warm, 0.0)
                nc.scalar.activation(warm, warm, EXP)

                # ------ Q, K projections: [CT][128, N] bf16, [c, n] layout
                def project_one(ws, b_sb, name, ot):
                    qt = persist.tile(
                        [P, N], BF16, tag=f"{name}_{ot}", name=f"{name}_{ot}"
                    )
                    for nh in range(2):
                        ps = pp.tile(
                            [P, 512], F32, tag="pp", name=f"pp_{name}{ot}{nh}"
                        )
                        for ct in range(CT):
                            nc.tensor.matmul(
                                ps,
                                lhsT=ws[ct][:, ot * P : (ot + 1) * P],
                                rhs=xs16[ct][:, nh * 512 : (nh + 1) * 512],
                                start=(ct == 0),
                                stop=(ct == CT - 1),
                            )
                        nc.vector.tensor_scalar_add(
                            qt[:, nh * 512 : (nh + 1) * 512],
                            ps,
                            b_sb[:, ot : ot + 1],
                        )
                    return qt

                Q, K = [], []
                Q.append(project_one(wqs, bq_sb, "q", 0))
                K.append(project_one(wks, bk_sb, "k", 0))

                # pair 0's S^T + exp right away: gets ScalarE going while
                # the remaining projections stream on the PE
                P16_0 = s_phase(0)
                Q.append(project_one(wqs, bq_sb, "q", 1))
                K.append(project_one(wks, bk_sb, "k", 1))
                P16_1 = s_phase(1)

                # ------ V^T projection: VT[jt] = [128, NH, HD+1] bf16
                wvs = load_w(wvd, "wv")
                bvB = persist.tile([P, DIM], F32, tag="bvB", name="bvB")
                nc.gpsimd.dma_start(
                    out=bvB,
                    in_=bass.AP(
                        tensor=bvd[:].tensor, offset=0, ap=[[0, P], [1, DIM]]
                    ),
                )
                VT = []
                for jt in range(JT):
                    vt = persist.tile(
                        [P, NH, HD + 1], BF16, tag=f"vt_{jt}", name=f"vt_{jt}"
                    )
                    ps = pp.tile([P, 512], F32, tag="pp", name=f"pp_v{jt}")
                    for ct in range(CT):
                        nc.tensor.matmul(
                            ps,
                            lhsT=xs16[ct][:, jt * P : (jt + 1) * P],
                            rhs=wvs[ct],
                            start=(ct == 0),
                            stop=(ct == CT - 1),
                        )
                    nc.vector.tensor_tensor(
                        vt[:, :, 0:HD],
                        ps.rearrange("p (h d) -> p h d", h=NH),
                        bvB.rearrange("p (h d) -> p h d", h=NH),
                        AOP.add,
                    )
                    nc.vector.memset(vt[:, :, HD : HD + 1], 1.0)
                    VT.append(vt)

                for ot in range(2, CT):
                    Q.append(project_one(wqs, bq_sb, "q", ot))
                    K.append(project_one(wks, bk_sb, "k", ot))

            # ---------- attention (heads 2p / 2p+1 live on partitions
            # 0-63 / 64-127 of Q/K c-tile p); the AV-accumulator pool
            # reuses banks the projection pool just released
            O16 = [
                persist.tile([P, N], BF16, tag=f"o16_{t}", name=f"o16_{t}")
                for t in range(CT)
            ]
            rdram = dram.tile([NH, N], F32, tag="rdram", name="rdram")
            with tc.tile_pool(name="psO", bufs=4, space="PSUM") as psO_pool:
                av_phase(0, P16_0, psO_pool, O16, rdram)
                P16_2 = s_phase(2)
                av_phase(1, P16_1, psO_pool, O16, rdram)
                P16_3 = s_phase(3)
                av_phase(2, P16_2, psO_pool, O16, rdram)
                av_phase(3, P16_3, psO_pool, O16, rdram)

                # loads for the output projection (low priority; the DMA
                # queues have slack mid-kernel)
                wos = load_w(wod, "wo")
                bo_sb = load_b(bod, "bo")
                x32r = x32d.rearrange("(t p) n -> t p n", p=P)
                xs32 = []
                for t in range(CT):
                    xt32 = persist.tile(
                        [P, N], F32, tag=f"x32_{t}", name=f"x32_{t}"
                    )
                    nc.sync.dma_start(out=xt32, in_=x32r[t])
                    nc.vector.tensor_scalar_add(xt32, xt32, bo_sb[:, t : t + 1])
                    xs32.append(xt32)

            # ---------- output projection + residual. ot0/ot1 psum tiles
            # come from the psS pool (slots drained by pair-3 exps);
            # ot2/ot3 from a pool reusing the psO banks (drained by the
            # early PSUM copies) — all 24 ct0-2 matmuls can therefore run
            # while the last head's epilogue is still in flight.
            with tc.tile_pool(name="po3", bufs=2, space="PSUM") as po3:
                outr = outd.rearrange("(t p) n -> t p n", p=P)

                def op_pre(ot, pool=None):
                    # ct 0..2 accumulation: issuable while the last head
                    # pair (feeding O16[3]) is still in its epilogue
                    if pool is None:
                        ps = psS_pool.tile([P, N], F32, tag="psS", name=f"ps_o{ot}")
                    else:
                        ps = pool.tile([P, N], F32, tag="op34", name=f"ps_o{ot}")
                    for nh in range(2):
                        for ct in range(CT - 1):
                            nc.tensor.matmul(
                                ps[:, nh * 512 : (nh + 1) * 512],
                                lhsT=wos[ct][:, ot * P : (ot + 1) * P],
                                rhs=O16[ct][:, nh * 512 : (nh + 1) * 512],
                                start=(ct == 0),
                                stop=(ct == CT - 2),
                            )
                    return ps

                def op_post(ot, ps):
                    # ct 3 continues the accumulation in a second group,
                    # then bias+residual and writeback
                    for nh in range(2):
                        nc.tensor.matmul(
                            ps[:, nh * 512 : (nh + 1) * 512],
                            lhsT=wos[CT - 1][:, ot * P : (ot + 1) * P],
                            rhs=O16[CT - 1][:, nh * 512 : (nh + 1) * 512],
                            start=False,
                            stop=True,
                            skip_group_check=True,
                        )
                    for nh in range(2):
                        ob = otile.tile([P, 512], F32, tag="ob", name="ob")
                        nc.vector.tensor_tensor(
                            ob,
                            ps[:, nh * 512 : (nh + 1) * 512],
                            xs32[ot][:, nh * 512 : (nh + 1) * 512],
                            AOP.add,
                        )
                        nc.sync.dma_start(
                            out=outr[ot][:, nh * 512 : (nh + 1) * 512], in_=ob
                        )

                ps0 = op_pre(0)
                ps1 = op_pre(1)
                ps2 = op_pre(2, po3)
                ps3 = op_pre(3, po3)
                op_post(0, ps0)
                op_post(1, ps1)
                op_post(2, ps2)
                op_post(3, ps3)
    return nc


_BF = ml_dtypes.bfloat16


def _prep_maps(x, Wq, bq, Wk, bk, Wv, bv, Wo, bo):
    # plain numpy up front: inputs may arrive as jax device arrays and
    # transforming those would trigger on-device jax execution
    x, Wq, bq, Wk, bk, Wv, bv, Wo, bo = (
        np.asarray(a) for a in (x, Wq, bq, Wk, bk, Wv, bv, Wo, bo)
    )
    B, C, H, W = x.shape
    xf = np.ascontiguousarray(x.reshape(B, C, H * W)).astype(np.float32)
    shared = {
        "wqt": np.ascontiguousarray(Wq.T).astype(_BF),
        "wkt": np.ascontiguousarray(Wk.T).astype(_BF),
        "wvt": np.ascontiguousarray(Wv.T).astype(_BF),
        "wot": np.ascontiguousarray(Wo.T).astype(_BF),
        "bq": np.asarray(bq, np.float32),
        "bk": np.asarray(bk, np.float32),
        "bv": np.asarray(bv, np.float32),
        "bo": np.asarray(bo, np.float32),
    }
    in_maps = []
    for b in range(B):
        m = dict(shared)
        m["x32"] = xf[b]
        m["x16"] = xf[b].astype(_BF)
        in_maps.append(m)
    return in_maps


def kernel(x, Wq, bq, Wk, bk, Wv, bv, Wo, bo, _trace=False):
    from concourse.bass_utils import run_bass_kernel_spmd

    x = np.asarray(x)
    B, C, H, W = x.shape
    in_maps = _prep_maps(x, Wq, bq, Wk, bk, Wv, bv, Wo, bo)
    nc = build_nc()
    res = run_bass_kernel_spmd(nc, in_maps, core_ids=list(range(B)), trace=_trace)
    out = np.stack([res.results[b]["out"] for b in range(B)])
    out = out.reshape(B, C, H, W).astype(np.float32)
    if _trace:
        kernel.last_results = res
    return out
